# revision 20
# baseline (speedup 1.0000x reference)
"""CliffordLinearSimple on 8 Trainium2 NeuronCores.

Math (per reference):
    sv   = x[:, :, SV_IDX]                      # [B, IN_F, 9]  (scalar+vector slots)
    svo  = sv.reshape(B, IN_F*9) @ W.T + b      # [B, OUT_F*9]
    v    = svo.reshape(B, OUT_F, 9)[:, :, 1:]   # [B, OUT_F, 8]
    biv  = v[:, :, IU] * v[:, :, JU]            # [B, OUT_F, 28]
    out[..., SV_IDX] = svo; out[..., BIV_IDX] = biv; rest 0

Distribution: tensor-parallel over OUT_F (row-split W): core c owns out
features [c*128, (c+1)*128).  Every core gets the full sv (gathered and
transposed on host -- only 9/256 of x's last dim is ever read), its W row
shard (pre-transposed to the PE's [K, N] layout and cast to bf16 on host),
and its bias shard.  Each core computes its [256, 128, 37] compact output
(9 linear slots + 28 bivector products) in bf16; the host upcasts and
scatters the compact slots into the zero-filled f32 [256,1024,256] output.

Matmul runs in bf16 (PE 1 cycle/row vs 4 for fp32; halves W HBM traffic);
PSUM accumulation is fp32.

Schedule (v2): the ~16 shared DMA engines deliver ~400 GB/s aggregate
across all queues, while PE W-consumption is ~307 GB/s and svT adds
~180 GB/s if co-streamed with a single 432-wide n-tile (the v1 n-major
schedule) -- oversubscribed, which showed up as ~14us of PE stalls.
v2 is k-major for the first 60 k-tiles: each k-group's matmuls cover all
three n-tiles (432+504+216 = 1152 cols), so svT delivery spreads over the
whole section at ~68 GB/s and total demand ~375 GB/s fits.  The last 12
k-tiles run tile-by-tile (n-major) so the three PSUM groups close at
staggered times: each tile's bias matmul (ones x b, stop=True), DVE
bivector scatter and SWDGE writeout overlap the next tile's matmuls, and
the kernel tail is only the 216-wide tile's epilogue.

Bias is accumulated at the END of each PSUM group instead of the start,
keeping the tiny bias DMA off the startup critical path; the k=0 matmuls
carry start=True.  Warm-up matmuls (junk data from the `ones` tile, which
gpsimd memsets immediately) release the HAM clock gate while the first
W/svT blocks are still in flight.  Bivector products run on DVE in bf16
(2x DVE rate), reading the PSUM->SBUF bf16 copy of the 9 sv slots; the 28
pair products collapse into 7 strided tensor_muls grouped by distance
d=j-i.  Compact outputs leave via SWDGE (gpsimd) as bf16.
"""
import sys

if "/opt/trn_rl_repo" not in sys.path:
    sys.path.insert(0, "/opt/trn_rl_repo")

from contextlib import ExitStack

import ml_dtypes
import numpy as np

import concourse.bass as bass
import concourse.tile as tile
from concourse import bacc, mybir
from concourse.bass_utils import run_bass_kernel_spmd

ALG_DIM = 8
D1 = 9
MV_DIM = 256
B, IN_F, OUT_F = 256, 1024, 1024
POW2 = np.array([2 ** i for i in range(ALG_DIM)])
SV_IDX = np.concatenate([[0], POW2])
IU, JU = np.triu_indices(ALG_DIM, 1)
BIV_IDX = POW2[IU] + POW2[JU]
NPAIR = len(IU)  # 28
NSLOT = 1 + ALG_DIM + NPAIR  # 37
NCORES = 8
OF = OUT_F // NCORES  # 128 out features per core

# bivector pairs grouped by distance d = j - i: one strided mul per group
PAIRS_BY_D = [(d, [(i, i + d) for i in range(ALG_DIM - d)]) for d in range(1, ALG_DIM)]
# compact-slot order: 9 sv slots, then the d-grouped pair products
IDX37 = list(SV_IDX) + [POW2[i] + POW2[j] for d, prs in PAIRS_BY_D for (i, j) in prs]
IDX37 = np.array(IDX37)
assert len(set(IDX37.tolist())) == NSLOT

# K = IN_F*9 = 9216 = KT*128.  KTLS: k-group sizes (in 128-deep k-tiles),
# identical for all three n-tiles.  The first NGA groups form the k-major
# section (all n-tiles interleaved per group); the last NGB groups run
# tile-by-tile.  Small leading groups start the PE pipeline early.
KTLS = (1, 2, 5, 6, 8, 8, 8, 8, 8, 6, 6, 6)
NGA = 10  # k-major groups (sum 60 k-tiles)
NGB = 2   # per-tile tail groups (sum 12 k-tiles)
FULL_CFG = dict(KT=72, KTLS=KTLS, NGA=NGA, OF=128, NTILES=(432, 504, 216), BT=2,
                WARM=12, WBUFS=12)


def build_core_program(KT, KTLS, NGA, OF, NTILES, BT, WARM=0, WBUFS=12):
    """SPMD per-core program: C[128*BT, OF*9] = svT.T @ Wh + b, then the
    9-slot copy + 28 bivector products into the compact bf16 output."""
    assert KT == sum(KTLS) and sum(NTILES) == OF * D1
    assert all(nt % D1 == 0 and nt <= 512 for nt in NTILES)
    NT = len(NTILES)
    NOFF = [sum(NTILES[:i]) for i in range(NT)]  # column offsets
    KOFFS = [sum(KTLS[:i]) for i in range(len(KTLS))]  # k-group offsets
    Bfull = BT * 128
    f32, bf16 = mybir.dt.float32, mybir.dt.bfloat16

    nc = bacc.Bacc("TRN2", target_bir_lowering=False, debug=False)
    svT_d = nc.dram_tensor("svT", [128, KT, Bfull], bf16, kind="ExternalInput").ap()
    # flat per-n W: k-group blocks [128, ktl, NTILE] packed contiguously in
    # group order, so every DMA reads one fully-sequential DRAM region
    W_ds = [
        nc.dram_tensor(f"Wh{n}", [KT * 128 * NTILES[n]], bf16, kind="ExternalInput").ap()
        for n in range(NT)
    ]
    b_d = nc.dram_tensor("bh", [1, OF * D1], bf16, kind="ExternalInput").ap()
    # flat chunk-major output: chunk (n, m) is one contiguous [128, NSLOT, och]
    # slab, so every SWDGE writeout is a single fully-sequential DRAM region;
    # the host reassembles (cheap transpose of the slot/feature axes)
    CHOFF = {}
    off = 0
    for n in range(NT):
        for m in range(BT):
            CHOFF[(n, m)] = off
            off += 128 * NSLOT * (NTILES[n] // D1)
    out_d = nc.dram_tensor("outc", [off], bf16, kind="ExternalOutput").ap()

    rings = [nc.sync, nc.scalar]  # the two HWDGE rings

    with tile.TileContext(nc) as tc:
        with ExitStack() as ctx:
            const = ctx.enter_context(tc.tile_pool(name="const", bufs=1))
            wpool = ctx.enter_context(tc.tile_pool(name="wpool", bufs=WBUFS))
            spool = ctx.enter_context(tc.tile_pool(name="spool", bufs=NT * BT))
            pspool = ctx.enter_context(
                tc.tile_pool(name="pspool", bufs=NT * BT, space="PSUM")
            )

            svT = const.tile([128, KT, Bfull], bf16)
            b_sb = const.tile([1, OF * D1], bf16)
            ones = const.tile([1, 512], bf16)
            # gpsimd memset: no dependence on the (busy) DMA-issuing engines
            # or on DVE; `ones` doubles as junk warm-up operands
            nc.gpsimd.memset(ones[:], 1.0)
            rings[1].dma_start(b_sb[:], b_d)
            # section B's svT chunks ride the otherwise-idle SWDGE queue,
            # issued up front: ~0.8MB with ~70us of deadline slack, and zero
            # displacement of the tightly-scheduled HWDGE input streams
            kb = KOFFS[NGA]
            nc.gpsimd.dma_start(svT[:, kb:KT, :], svT_d[:, kb:KT, :])

            # all PSUM accumulators live for the whole kernel (NT*BT banks)
            ps = {
                (m, n): pspool.tile([128, NTILES[n]], f32, name=f"ps{m}_{n}", tag="ps")
                for n in range(NT)
                for m in range(BT)
            }

            # PE warm-up with no DMA deps: junk matmuls into ps[0,0] (its
            # k=0 matmul below re-opens the bank with start=True), so the
            # HAM clock gate is already ramping when real work arrives.
            # 432-wide: long enough that warm-up outlasts the first W/svT
            # blocks' arrival even if the clock ramps mid-warm -- a PE idle
            # gap >~2us resets the ~7.7us continuous-busy HAM ramp timer.
            for _ in range(WARM):
                nc.tensor.matmul(
                    ps[(0, 0)][:], ones[:, 0:128], ones[:, 0:NTILES[0]],
                    start=True, stop=True, skip_group_check=True,
                )

            def w_dma(n, g, ring):
                ktl = KTLS[g]
                k0 = KOFFS[g]
                wt = wpool.tile([128, ktl, NTILES[n]], bf16, name="wt", tag="wt")
                blk = W_ds[n][k0 * 128 * NTILES[n]:(k0 + ktl) * 128 * NTILES[n]]
                ring.dma_start(wt[:], blk.rearrange("(p r) -> p r", p=128))
                return wt

            # ---- section A: k-major over all three n-tiles (groups 0..NGA) ----
            # per-group ring split is {svT, W1} vs {W0, W2} (194 vs 166 KB per
            # k-tile), alternating sides each group so both rings average 50%
            # of the byte stream -- the DMA engines serve the two queues about
            # evenly, so a persistently heavier ring falls steadily behind and
            # stalls the PE at its semaphores.
            for g in range(NGA):
                ktl, k0 = KTLS[g], KOFFS[g]
                ra, rb = (rings[0], rings[1]) if g % 2 == 0 else (rings[1], rings[0])
                ra.dma_start(svT[:, k0:k0 + ktl, :], svT_d[:, k0:k0 + ktl, :])
                wts = [
                    w_dma(0, g, rb),
                    w_dma(1, g, ra),
                    w_dma(2, g, rb),
                ]
                for ki in range(ktl):
                    kt = k0 + ki
                    for m in range(BT):
                        for n in range(NT):
                            nc.tensor.matmul(
                                ps[(m, n)][:],
                                svT[:, kt, m * 128:(m + 1) * 128],
                                wts[n][:, ki],
                                start=(kt == 0),
                                stop=False,
                            )

            # ---- section B: per-tile tail groups; staggered epilogues ----
            # m runs OUTER so chunk (n, m=0) closes (and its DVE scatter
            # starts) while the PE is still sweeping m=1.
            ring_i = 0

            def next_ring():
                nonlocal ring_i
                ring_i ^= 1
                return rings[ring_i]

            # tile order (1, 0, 2): the widest tile's (largest) DVE chunks go
            # first so DVE is drained by the time the narrow final tile's
            # epilogue -- the kernel tail -- runs
            for n in (1, 0, 2):
                wts = [w_dma(n, g, next_ring()) for g in range(NGA, NGA + NGB)]
                och = NTILES[n] // D1
                for m in range(BT):
                    # bias accumulated up front (order within a PSUM group is
                    # free), so the group closes on the last k-matmul and the
                    # epilogue starts with zero extra PE work
                    nc.tensor.matmul(
                        ps[(m, n)][:],
                        ones[:, 0:128],
                        b_sb[:, NOFF[n]:NOFF[n] + NTILES[n]],
                        start=False,
                        stop=False,
                    )
                    for gi, g in enumerate(range(NGA, NGA + NGB)):
                        ktl, k0 = KTLS[g], KOFFS[g]
                        for ki in range(ktl):
                            kt = k0 + ki
                            nc.tensor.matmul(
                                ps[(m, n)][:],
                                svT[:, kt, m * 128:(m + 1) * 128],
                                wts[gi][:, ki],
                                start=False,
                                stop=(kt == KT - 1),
                            )
                    # chunk epilogue, pipelined across engines: the Activation
                    # engine casts the 9 sv slots PSUM->SBUF bf16 (Pool cannot
                    # read PSUM), DVE and Pool split the 7 distance-grouped
                    # pair products (slot-major st makes every mul contiguous
                    # bf16), and the idle sync HWDGE ring carries the slab out
                    # (SWDGE measured ~30 GB/s on these writes; HWDGE is ~10x)
                    psr = ps[(m, n)].rearrange("p (o j) -> p j o", j=D1)
                    st = spool.tile([128, NSLOT, och], bf16, name="st", tag="st")
                    nc.scalar.copy(st[:, 0:D1, :], psr[:])
                    s = D1
                    for d, prs in PAIRS_BY_D:
                        w = len(prs)  # pairs (i, i+d), i = 0..w-1
                        nc.vector.tensor_mul(
                            st[:, s:s + w, :],
                            st[:, 1:1 + w, :],
                            st[:, 1 + d:1 + d + w, :],
                        )
                        s += w
                    # writeout in two slices: slots [0,27) (ready after the
                    # d=3 mul) ship while DVE finishes d=4..7, hiding most of
                    # the DMA latency behind the remaining muls
                    co = CHOFF[(n, m)]
                    SPLIT = D1 + 7 + 6 + 5  # 27
                    flat = out_d[co:co + 128 * NSLOT * och].rearrange(
                        "(p r) -> p r", p=128
                    ).rearrange("p (s o) -> p s o", s=NSLOT)
                    rings[0].dma_start(flat[:, 0:SPLIT, :], st[:, 0:SPLIT, :])
                    rings[0].dma_start(flat[:, SPLIT:NSLOT, :], st[:, SPLIT:NSLOT, :])

    nc.finalize()
    return nc


_PROGRAM = None


def _get_program():
    global _PROGRAM
    if _PROGRAM is None:
        _PROGRAM = build_core_program(**FULL_CFG)
    return _PROGRAM


def _prep_inputs(x, W, b):
    bf16 = ml_dtypes.bfloat16
    KT, NTILES = FULL_CFG["KT"], FULL_CFG["NTILES"]
    NOFF = [sum(NTILES[:i]) for i in range(len(NTILES))]
    # svT[p, kt, m] = sv[m, kt*128 + p], sv = x[:, :, SV_IDX] flattened
    sv = np.ascontiguousarray(x[:, :, SV_IDX]).reshape(B, IN_F * D1)
    svT = np.ascontiguousarray(sv.reshape(B, KT, 128).transpose(2, 1, 0)).astype(bf16)

    Wb = W.astype(bf16)
    # Wr[c, o', kt, p] with o' the core-local output column
    Wr = Wb.reshape(NCORES, OF * D1, KT, 128)
    KTLS = FULL_CFG["KTLS"]
    KOFFS = [sum(KTLS[:i]) for i in range(len(KTLS))]
    in_maps = []
    for c in range(NCORES):
        m = {
            "svT": svT,
            "bh": np.ascontiguousarray(b[c * OF * D1:(c + 1) * OF * D1]).astype(bf16).reshape(1, OF * D1),
        }
        for n, nt in enumerate(NTILES):
            # per k-group block [p, ktl, jj] = W_core[NOFF[n]+jj, kt*128+p],
            # raveled + concatenated (matches the device-side slices)
            sub = Wr[c, NOFF[n]:NOFF[n] + nt]  # [jj, kt, p]
            parts = []
            for g, ktl in enumerate(KTLS):
                a = KOFFS[g]
                blk = sub[:, a:a + ktl]  # [jj, ktl, p]
                parts.append(np.ascontiguousarray(blk.transpose(2, 1, 0)).ravel())
            m[f"Wh{n}"] = np.concatenate(parts)
        in_maps.append(m)
    return in_maps


def run(x, W, b, trace=False):
    x = np.asarray(x, dtype=np.float32)
    W = np.asarray(W, dtype=np.float32)
    b = np.asarray(b, dtype=np.float32)
    in_maps = _prep_inputs(x, W, b)
    nc = _get_program()
    res = None
    for attempt in range(3):
        try:
            res = run_bass_kernel_spmd(
                nc, in_maps, core_ids=list(range(NCORES)), trace=trace
            )
            break
        except Exception:
            if attempt == 2:
                raise
            import time as _time
            _time.sleep(5)
    NTILES, BT = FULL_CFG["NTILES"], FULL_CFG["BT"]
    NOFF = [sum(NTILES[:i]) for i in range(len(NTILES))]
    out = np.zeros((B, OUT_F, MV_DIM), dtype=np.float32)
    comp = np.empty((B, OUT_F, NSLOT), dtype=np.float32)
    for c in range(NCORES):
        flat = np.asarray(res.results[c]["outc"])
        off = 0
        for n, nt in enumerate(NTILES):
            och = nt // D1
            for m in range(BT):
                slab = flat[off:off + 128 * NSLOT * och].reshape(128, NSLOT, och)
                comp[m * 128:(m + 1) * 128, c * OF + NOFF[n] // D1:c * OF + NOFF[n] // D1 + och, :] = (
                    slab.transpose(0, 2, 1).astype(np.float32)
                )
                off += 128 * NSLOT * och
    out[:, :, IDX37] = comp
    return out, res


def kernel(x, W, b):
    out, _ = run(x, W, b)
    return out
